# revision 1
# baseline (speedup 1.0000x reference)
"""DDGCRN cell on 8 TRN2 NeuronCores — data-parallel over batch.

Per core: 8 batches = 16 branch-instances (gate O=128 / update O=64), emitted
as a software pipeline so every engine's static instruction stream stays
dense (engines execute their streams in order; serial per-instance chains
would otherwise stall the TensorEngine and re-throttle its HAM clock gate).

Pipeline: step s emits  P1(s+1) hypernet layers INTERLEAVED with P2(s)
A-chunks (the PE always has A-matmuls queued while Act produces the next
sigmoid) | P6(s-2) op-matmuls+activation | P4(s-1) yT+yd | P5(s-1) Lx parts
+ packed-Z build | P3(s) rsqrt+x'+d-row | glue(s-2) at step end (so the
PSUM ring recycles through Act-read slots). Update(b) >= 4 slots after
gate(b). PSUM accumulators are single 2-bank tiles: each column split gets
its own matmul (bank-limited) but relu/yd/zout consume the full 883-wide
tile in ONE instruction. Work is spread over all four compute engines
(gpsimd takes half the Z-muls, the cand-natural builds and the epilogue;
Act takes the d*x scaling via activation-scale).

Key layout trick vs the earlier version: features are permuted state-first
([64 state; 2 x]), so the final per-node einsum packs its contraction to the
full 128 PE rows:  out^T = bp.T@embT (K=10) + WX.T@zx40 (K=40, the 2 x-feats
x {cheb k} x {10 e}) + sum_e WZS_e.T @ (embSS_e * S2) (10 matmuls, K=128,
S2 = [Lx-state(64); x-state(64)]) — 12 x 883 moving cols instead of 21 x 883.
Engine partition-shift rule honored throughout: every operand of a compute
op shares one (in_base - out_base) in {0,32,64,96}.

Math per instance:
  filt = hypernet MLP (transposed-feature layout, bf16)
  V = tanh(emb*time*day*speed*occupy*filt)      (10, 883)
  A = relu(V V^T) (883,883 symmetric) + fused row-sums (ACT accum_out)
  d = rsqrt(rowsum) via fast-inverse-sqrt + 1 Newton step (DVE only)
  y^T = (d*xs)^T A  (A symmetric); yd = y^T * (ones x d_row); Lx = x0 - yd
  out^T = packed einsum above (PSUM accumulation; bias folded)

All matmuls bf16 (PSUM f32); inputs pre-cast/pre-transposed on host (pure
layout/dtype prep). Output written transposed bf16, un-transposed on host.
"""

import sys, os

sys.path.insert(0, "/opt/trn_rl_repo")

import numpy as np
import ml_dtypes
from contextlib import ExitStack

import concourse.bass as bass
import concourse.bacc as bacc
import concourse.mybir as mybir
from concourse import tile
from concourse.alu_op_type import AluOpType
from concourse.bass_utils import run_bass_kernel_spmd

AF = mybir.ActivationFunctionType
F32 = mybir.dt.float32
BF16 = mybir.dt.bfloat16
I32 = mybir.dt.int32
BF16_NP = ml_dtypes.bfloat16

B, N, DIN, DOUT, E, CHEB = 64, 883, 2, 64, 10, 2
C = DIN + DOUT  # 66
NCORES = 8
BL = B // NCORES  # 8 batches per core
NT = (N + 127) // 128  # 7 row tiles
OG, OU = 2 * DOUT, DOUT  # 128, 64
SPLITS = [(0, 512), (512, N - 512)]
RSQRT_MAGIC = 0x5F3759DF

# instance schedule: update(b) >= 4 slots after gate(b) (P1 is emitted a full
# step early, so glue(gate) must precede P1(update) by at least one step)
SEQ = [("g", 0), ("g", 1), ("g", 2), ("g", 3), ("u", 0), ("g", 4), ("u", 1),
       ("g", 5), ("u", 2), ("g", 6), ("u", 3), ("g", 7), ("u", 4), ("u", 5),
       ("u", 6), ("u", 7)]


def _pt(nt):
    return min(128, N - nt * 128)


def _build_body(tc, ctx, nc, P):
    def pool(name, bufs, space="SBUF"):
        return ctx.enter_context(tc.tile_pool(name=name, bufs=bufs, space=space))

    wp = pool("wp", 1)        # static weights
    dat = pool("dat", 2)      # per-batch DMA loads
    act = pool("act", 2)      # per-instance intermediates
    arp = pool("arp", 15)     # relu(A) tiles: 2 instances x 7 in flight
    xnp = pool("xnp", 22)     # natural xs/cand tiles
    xpp = pool("xpp", 15)     # d*xs tiles
    zp = pool("zp", 10)       # packed Z tiles (128, N)
    dnp = pool("dnp", 4)      # rowsum/d helpers
    psp = pool("psp", 2, space="PSUM")  # op + yT accumulators (2-bank tiles)
    psa = pool("psa", 2, space="PSUM")  # A / hypernet / misc (2-bank tiles)

    # ---------------- static setup ----------------
    ident_f = wp.tile([128, 128], F32, tag="identf", name="ident_f")
    nc.sync.dma_start(ident_f[:, :], P["ident"][:, :])
    identB64 = wp.tile([128, 64], BF16, tag="identb64", name="identB64")
    nc.sync.dma_start(identB64[:, :], P["identB64"][:, :])
    ones66 = wp.tile([1, C], BF16, tag="ones66", name="ones66")
    nc.vector.memset(ones66[:, :], 1.0)

    def load_bf(pname, shape, tag):
        t = wp.tile(list(shape), BF16, tag=tag, name=pname + "_t")
        nc.sync.dma_start(t[:, :], P[pname][:, :])
        return t

    embT = load_bf("embT", (E, N), "embT")
    embX40 = load_bf("embX40", (40, N), "embX40")
    sel4 = load_bf("sel4", (4, 40), "sel4")
    embSS = []
    for e in range(E):
        t = wp.tile([128, N], BF16, tag=f"embSS{e}", name=f"embSS{e}")
        nc.sync.dma_start(t[:, :], P["embSS"][e * 128:(e + 1) * 128, :])
        embSS.append(t)
    wzs = {}
    for br, On in (("g", OG), ("u", OU)):
        tiles = []
        for e in range(E):
            t = wp.tile([128, On], BF16, tag=f"wzs{br}{e}", name=f"wzs{br}{e}")
            nc.sync.dma_start(t[:, :], P[f"wzs_{br}"][e * 128:(e + 1) * 128, :])
            tiles.append(t)
        wzs[br] = tiles
    wx = {"g": load_bf("wx_g", (40, OG), "wxg"),
          "u": load_bf("wx_u", (40, OU), "wxu")}
    bp = {"g": load_bf("bpool_g", (E, OG), "bpg"),
          "u": load_bf("bpool_u", (E, OU), "bpu")}
    fc = {}
    for br in ("g", "u"):
        fc[("w1", br)] = load_bf(f"fc1w_{br}", (C, 16), f"fc1w{br}")
        fc[("w2", br)] = load_bf(f"fc2w_{br}", (16, 2), f"fc2w{br}")
        fc[("w3", br)] = load_bf(f"fc3w_{br}", (2, E), f"fc3w{br}")
        for nm, shape in (("b1", (16, 1)), ("b2", (2, 1)), ("b3", (E, 1))):
            t = wp.tile(list(shape), F32, tag=f"fc{nm}{br}", name=f"fc{nm}{br}")
            nc.sync.dma_start(t[:, :], P[f"fc{nm}_{br}"][:, :])
            fc[(nm, br)] = t

    # ---------------- per-instance state ----------------
    ST = {}   # (br,b) -> dict of tiles
    BAT = {}  # b -> dict of per-batch tiles

    def batch_load(b):
        """DMA this batch's inputs; build Mb."""
        d = {}
        xs_nat = []
        for nt in range(NT):
            p = _pt(nt)
            t = xnp.tile([128, C], BF16, tag="xsn", name=f"xsn{b}{nt}")
            nc.sync.dma_start(t[:p, 0:DOUT], P["state_nat"][b, nt * 128:nt * 128 + p, :])
            nc.sync.dma_start(t[:p, DOUT:C], P["x_nat"][b, nt * 128:nt * 128 + p, :])
            xs_nat.append(t)
        d["xs_nat"] = xs_nat
        # csT: rows 0..63 state^T, rows 64..65 x^T  (gate hypernet moving;
        # base-0 state for subs/epilogue; x rows at base 64)
        csT = act.tile([C, N], BF16, tag="csT", name=f"csT{b}", bufs=6)
        nc.sync.dma_start(csT[:, :], P["csT"][b, :, :])
        d["csT"] = csT
        # S2g: rows 64..127 = state^T (k=0 block); rows 0..63 filled in P5(g)
        S2g = act.tile([128, N], BF16, tag="S2g", name=f"S2g{b}", bufs=4)
        nc.sync.dma_start(S2g[64:128, :], P["stateT"][b, :, :])
        d["S2g"] = S2g
        tdso = []
        for nm in ("tT", "dT", "sT", "oT"):
            t = dat.tile([E, N], BF16, tag=nm, name=f"{nm}{b}", bufs=2)
            nc.sync.dma_start(t[:, :], P[nm][b, :, :])
            tdso.append(t)
        p1 = act.tile([E, N], BF16, tag="p1", name=f"p1_{b}", bufs=2)
        nc.vector.tensor_mul(p1[:, :], tdso[0][:, :], tdso[1][:, :])
        p2 = act.tile([E, N], BF16, tag="p2", name=f"p2_{b}", bufs=1)
        nc.gpsimd.tensor_mul(p2[:, :], tdso[2][:, :], tdso[3][:, :])
        p3 = act.tile([E, N], BF16, tag="p1", name=f"p3_{b}", bufs=2)
        nc.vector.tensor_mul(p3[:, :], p1[:, :], p2[:, :])
        Mb = act.tile([E, N], BF16, tag="Mb", name=f"Mb{b}", bufs=5)
        nc.vector.tensor_mul(Mb[:, :], p3[:, :], embT[:, :])
        d["Mb"] = Mb
        BAT[b] = d

    def P1_load(inst):
        """Batch load + per-instance init (rs accumulators)."""
        br, b = inst
        if br == "g":
            batch_load(b)
            st = ST[inst] = {}
            st["x0T"] = BAT[b]["csT"]
        else:
            st = ST[inst]  # created by glue(gate): has x0T=CU
        rs = dnp.tile([128, 8], F32, tag="rs0", name=f"rs0{br}{b}")
        nc.vector.memset(rs[:, :], 1.0)
        st["rs"] = rs

    def P1_l1(inst):
        br, b = inst
        st = ST[inst]
        xg2 = st["x0T"]
        h1p = psa.tile([16, 896], F32, tag="psB", name=f"h1p{br}{b}")
        h1 = act.tile([16, N], BF16, tag="h1", name=f"h1{br}{b}")
        for s0, sl in SPLITS:
            nc.tensor.matmul(h1p[:16, s0:s0 + sl], fc[("w1", br)][:, :],
                             xg2[:, s0:s0 + sl], start=True, stop=True)
        nc.scalar.activation(h1[:, :], h1p[:16, 0:N],
                             AF.Sigmoid, bias=fc[("b1", br)][:, :])
        st["h1"] = h1

    def P1_l2(inst):
        br, b = inst
        st = ST[inst]
        h2p = psa.tile([2, 896], F32, tag="psB", name=f"h2p{br}{b}")
        h2 = act.tile([2, N], BF16, tag="h2", name=f"h2{br}{b}")
        for s0, sl in SPLITS:
            nc.tensor.matmul(h2p[:2, s0:s0 + sl], fc[("w2", br)][:, :],
                             st["h1"][:, s0:s0 + sl], start=True, stop=True)
        nc.scalar.activation(h2[:, :], h2p[:2, 0:N],
                             AF.Sigmoid, bias=fc[("b2", br)][:, :])
        st["h2"] = h2

    def P1_l3V(inst):
        br, b = inst
        st = ST[inst]
        h3p = psa.tile([E, 896], F32, tag="psB", name=f"h3p{br}{b}")
        filt = act.tile([E, N], BF16, tag="filt", name=f"filt{br}{b}")
        for s0, sl in SPLITS:
            nc.tensor.matmul(h3p[:E, s0:s0 + sl], fc[("w3", br)][:, :],
                             st["h2"][:, s0:s0 + sl], start=True, stop=True)
        nc.scalar.activation(filt[:, :], h3p[:E, 0:N],
                             AF.Identity, bias=fc[("b3", br)][:, :])
        vpre = act.tile([E, N], BF16, tag="vpre", name=f"vpre{br}{b}")
        nc.vector.tensor_mul(vpre[:, :], BAT[b]["Mb"][:, :], filt[:, :])
        V = act.tile([E, N], BF16, tag="V", name=f"V{br}{b}")
        nc.scalar.activation(V[:, :], vpre[:, :], AF.Tanh)
        st["V"] = V

    def P2_chunk(inst, kts):
        """A = relu(V V^T) + fused row-sums, for a subset of row tiles."""
        br, b = inst
        st = ST[inst]
        V, rsh = st["V"], st["rs"]
        ar = st.setdefault("ar", [])
        for kt in kts:
            p = _pt(kt)
            aps = psa.tile([128, 896], F32, tag="psB", name=f"aps{br}{b}{kt}")
            art = arp.tile([128, N], BF16, tag="ar", name=f"ar{br}{b}{kt}")
            for s0, sl in SPLITS:
                nc.tensor.matmul(aps[:p, s0:s0 + sl],
                                 V[:, kt * 128:kt * 128 + p],
                                 V[:, s0:s0 + sl], start=True, stop=True)
            nc.scalar.activation(art[:p, 0:N], aps[:p, 0:N],
                                 AF.Relu, accum_out=rsh[:p, kt:kt + 1])
            ar.append(art)

    def P3(inst):
        """d = rsqrt(rowsums) on DVE; d-row broadcast + dB; x' = d*xs."""
        br, b = inst
        st = ST[inst]
        rsall = st["rs"]
        tsh = dnp.tile([128, 8], F32, tag="tsh", name=f"tsh{br}{b}")
        nc.vector.tensor_scalar(tsh[:, :].bitcast(I32), rsall[:, :].bitcast(I32),
                                1, None, AluOpType.logical_shift_right)
        tnot = dnp.tile([128, 8], F32, tag="tnot", name=f"tnot{br}{b}")
        nc.vector.tensor_scalar(tnot[:, :].bitcast(I32), tsh[:, :].bitcast(I32),
                                -1, None, AluOpType.bitwise_xor)
        d0 = dnp.tile([128, 8], F32, tag="d0", name=f"d0{br}{b}")
        nc.vector.tensor_scalar(d0[:, :].bitcast(I32), tnot[:, :].bitcast(I32),
                                RSQRT_MAGIC + 1, None, AluOpType.add)
        sq = dnp.tile([128, 8], F32, tag="sq", name=f"sq{br}{b}")
        nc.vector.tensor_mul(sq[:, :], d0[:, :], d0[:, :])
        hx = dnp.tile([128, 8], F32, tag="hx", name=f"hx{br}{b}")
        nc.vector.tensor_mul(hx[:, :], sq[:, :], rsall[:, :])
        cf = dnp.tile([128, 8], F32, tag="cf", name=f"cf{br}{b}")
        nc.vector.tensor_scalar(cf[:, :], hx[:, :], -0.5, 1.5,
                                AluOpType.mult, AluOpType.add)
        dcat = dnp.tile([128, 8], F32, tag="dcat", name=f"dcat{br}{b}")
        nc.vector.tensor_mul(dcat[:, :], d0[:, :], cf[:, :])
        st["dcat"] = dcat
        # d-row broadcast: transpose -> flatten DMA -> ones-outer -> dB66 SBUF
        tp = psa.tile([128, 128], F32, tag="psB", name=f"dtp{br}{b}")
        nc.tensor.transpose(tp[:8, :128], dcat[:, :], ident_f[:, :])
        drs = act.tile([8, 128], BF16, tag="drs", name=f"drs{br}{b}")
        nc.scalar.copy(drs[:, :], tp[:8, :128])
        drow = act.tile([1, 1024], BF16, tag="drow", name=f"drow{br}{b}")
        nc.sync.dma_start(drow[0:1, :], drs[0:8, :])
        dB = act.tile([C, N], BF16, tag="dB", name=f"dB{br}{b}")
        dbp = psa.tile([C, 896], F32, tag="psB", name=f"dbp{br}{b}")
        for s0, sl in SPLITS:
            nc.tensor.matmul(dbp[:C, s0:s0 + sl], ones66[:, :],
                             drow[0:1, s0:s0 + sl], start=True, stop=True)
        nc.scalar.copy(dB[:, :], dbp[:C, 0:N])
        st["dB"] = dB
        # X4 tile (rows 2..3 = x^T via DMA; rows 0..1 = Lx-x, filled in P5)
        x4 = act.tile([4, 896], BF16, tag="x4", name=f"x4{br}{b}", bufs=4)
        nc.sync.dma_start(x4[2:4, 0:N], P["xT"][b, :, :])
        st["x4"] = x4
        xnat = BAT[b]["xs_nat"] if br == "g" else st["cn"]
        xp = []
        for kt in range(NT):
            p = _pt(kt)
            xpt = xpp.tile([128, C], BF16, tag="xp", name=f"xp{br}{b}{kt}")
            nc.scalar.activation(xpt[:p, :], xnat[kt][:p, :], AF.Identity,
                                 scale=dcat[:p, kt:kt + 1])
            xp.append(xpt)
        st["xp"] = xp

    def P4(inst):
        """y^T matmuls + yd = y^T * dB."""
        br, b = inst
        st = ST[inst]
        yt = psp.tile([C, 896], F32, tag="psA", name=f"yt{br}{b}")
        ar, xp = st["ar"], st["xp"]
        for kt in range(NT):
            p = _pt(kt)
            for s0, sl in SPLITS:
                nc.tensor.matmul(yt[:C, s0:s0 + sl], xp[kt][:p, :],
                                 ar[kt][:p, s0:s0 + sl],
                                 start=(kt == 0), stop=(kt == NT - 1))
        dB = st["dB"]
        yd = act.tile([C, N], BF16, tag="yd", name=f"yd{br}{b}")
        nc.vector.tensor_mul(yd[:, :], yt[:C, 0:N], dB[:, :])
        st["yd"] = yd

    def P5(inst):
        """Lx pieces (state at base 0, x rows) + packed moving operands."""
        br, b = inst
        st = ST[inst]
        yd = st["yd"]
        x0T = st["x0T"]                      # csT (gate) / CU (update)
        S2 = BAT[b]["S2g"] if br == "g" else st["S2u"]
        # S2 rows 0..63 = Lx-state = x0-state - yd-state  (all base 0)
        nc.vector.tensor_sub(S2[0:64, :], x0T[0:64, :], yd[0:64, :])
        # X4 rows 0..1 = Lx-x = x0-x - yd-x  (ins at base 64, out base 0)
        x4 = st["x4"]
        nc.vector.tensor_sub(x4[0:2, 0:N], x0T[64:66, :], yd[64:66, :])
        # xrep = SEL4^T X4 (40, sl) psum; zx40 = embX40 * xrep
        zx = act.tile([40, 896], BF16, tag="zx", name=f"zx{br}{b}", bufs=3)
        xrp = psa.tile([40, 896], F32, tag="psB", name=f"xrp{br}{b}")
        for s0, sl in SPLITS:
            nc.tensor.matmul(xrp[:40, s0:s0 + sl], sel4[:, :],
                             x4[0:4, s0:s0 + sl], start=True, stop=True)
        nc.vector.tensor_mul(zx[:, 0:N], embX40[:, :], xrp[:40, 0:N])
        st["zx"] = zx
        # packed Z moving tiles; last 4 built on the (otherwise idle) gpsimd
        zt = []
        for e in range(E):
            z = zp.tile([128, N], BF16, tag="Z", name=f"Z{br}{b}{e}", bufs=10)
            eng = nc.gpsimd if e >= 5 else nc.vector
            eng.tensor_mul(z[:, :], embSS[e][:, :], S2[:, :])
            zt.append(z)
        st["zt"] = zt

    def P6(inst):
        """Final per-node einsum (packed contraction) + output activation."""
        br, b = inst
        st = ST[inst]
        On = OG if br == "g" else OU
        outf = AF.Sigmoid if br == "g" else AF.Tanh
        op = psp.tile([On, 896], F32, tag="psA", name=f"op{br}{b}")
        zx, zt = st["zx"], st["zt"]
        for s0, sl in SPLITS:
            nc.tensor.matmul(op[:On, s0:s0 + sl], bp[br][:, :],
                             embT[:, s0:s0 + sl], start=True, stop=False)
            nc.tensor.matmul(op[:On, s0:s0 + sl], wx[br][:, :],
                             zx[:, s0:s0 + sl], start=False, stop=False)
            for e in range(E):
                nc.tensor.matmul(op[:On, s0:s0 + sl], wzs[br][e][:, :],
                                 zt[e][:, s0:s0 + sl], start=False,
                                 stop=(e == E - 1))
        zout = act.tile([On, N], BF16, tag=f"zout{br}",
                        name=f"zout{br}{b}", bufs=(5 if br == "g" else 2))
        nc.scalar.activation(zout[:, :], op[:On, 0:N], outf)
        st["zout"] = zout

    def glue(inst):
        """After P6: gate -> build update inputs; update -> epilogue + store."""
        br, b = inst
        if br == "g":
            zr = ST[inst]["zout"]  # (128, N): rows 0..63 = r, 64..127 = z
            S2g = BAT[b]["S2g"]    # rows 64..127 = state^T
            csT = BAT[b]["csT"]
            ust = {}
            ST[("u", b)] = ust
            # S2u rows 64..127 = cand-state = z * state (k=0 block)
            S2u = act.tile([128, N], BF16, tag="S2u", name=f"S2u{b}", bufs=3)
            nc.vector.tensor_mul(S2u[64:128, :], zr[64:128, :], S2g[64:128, :])
            ust["S2u"] = S2u
            # CU: update hypernet input, rows 0..63 cand-state, 64..65 x
            CU = act.tile([C, N], BF16, tag="CU", name=f"CU{b}", bufs=3)
            nc.vector.tensor_mul(CU[0:64, :], zr[64:128, :], S2g[64:128, :])
            nc.vector.tensor_copy(CU[64:66, :], csT[64:66, :])
            ust["x0T"] = CU
            # natural cand tiles for update's x' (d * cand)
            cn_l = []
            for nt in range(NT):
                p = _pt(nt)
                zps = psa.tile([128, 64], BF16, tag="psB", name=f"znp{b}{nt}")
                nc.tensor.transpose(zps[:p, :DOUT],
                                    zr[64:128, nt * 128:nt * 128 + p],
                                    identB64[64:128, :])
                zn = act.tile([128, DOUT], BF16, tag="zn", name=f"zn{b}{nt}",
                              bufs=4)
                nc.scalar.copy(zn[:p, :], zps[:p, :DOUT])
                cn = xnp.tile([128, C], BF16, tag="cn", name=f"cn{b}{nt}",
                              bufs=22)
                nc.gpsimd.tensor_mul(cn[:p, 0:DOUT], zn[:p, :],
                                     BAT[b]["xs_nat"][nt][:p, 0:DOUT])
                nc.gpsimd.tensor_copy(cn[:p, DOUT:C],
                                      BAT[b]["xs_nat"][nt][:p, DOUT:C])
                cn_l.append(cn)
            ust["cn"] = cn_l
        else:
            # epilogue on gpsimd: off the DVE/Act critical path, and the
            # result only feeds the output DMA.
            hc = ST[inst]["zout"]          # (64, N) at base 0
            r = ST[("g", b)]["zout"]       # gate zout rows 0..63 = r
            csT = BAT[b]["csT"]            # rows 0..63 = state^T (base 0)
            t1 = act.tile([OU, N], BF16, tag="t1", name=f"t1_{b}", bufs=2)
            nc.gpsimd.tensor_sub(t1[:, :], csT[0:64, :], hc[:, :])
            t2 = act.tile([OU, N], BF16, tag="t2", name=f"t2_{b}", bufs=2)
            nc.gpsimd.tensor_mul(t2[:, :], r[0:64, :], t1[:, :])
            outT = act.tile([OU, N], BF16, tag="outT", name=f"outT{b}")
            nc.gpsimd.tensor_add(outT[:, :], t2[:, :], hc[:, :])
            nc.sync.dma_start(P["out"][b, :, :], outT[:, :])

    # ---------------- pipeline driver ----------------
    # Step head interleaves P1(s+1) hypernet layers with P2(s) A-chunks: the
    # PE always has A-matmuls queued while Act produces the next sigmoid, so
    # the hypernet's PE<->Act ping-pong never idles the PE. P2 before P6 also
    # makes the A psum ring recycle Act-read slots instead of DVE-read ones.
    M = len(SEQ)
    P1_load(SEQ[0]); P1_l1(SEQ[0]); P1_l2(SEQ[0]); P1_l3V(SEQ[0])
    for s in range(M + 2):
        nxt = SEQ[s + 1] if s + 1 < M else None
        cur = SEQ[s] if s < M else None
        if nxt:
            P1_load(nxt)
            P1_l1(nxt)
        if cur:
            P2_chunk(cur, range(0, 4))
        if nxt:
            P1_l2(nxt)
        if cur:
            P2_chunk(cur, range(4, 7))
        if nxt:
            P1_l3V(nxt)
        if 0 <= s - 2 < M:
            P6(SEQ[s - 2])
        if 0 <= s - 1 < M:
            P4(SEQ[s - 1])
            P5(SEQ[s - 1])
        if cur:
            P3(cur)
        if 0 <= s - 2 < M:
            glue(SEQ[s - 2])


def build_nc():
    nc = bacc.Bacc()
    P = {}

    def dp(name, shape, dtype=F32, out=False):
        P[name] = nc.declare_dram_parameter(name, list(shape), dtype, isOutput=out)

    dp("x_nat", (BL, N, DIN), BF16)
    dp("state_nat", (BL, N, DOUT), BF16)
    dp("csT", (BL, C, N), BF16)
    dp("stateT", (BL, DOUT, N), BF16)
    dp("xT", (BL, DIN, N), BF16)
    for nm in ("tT", "dT", "sT", "oT"):
        dp(nm, (BL, E, N), BF16)
    dp("embT", (E, N), BF16)
    dp("embSS", (E * 128, N), BF16)
    dp("embX40", (40, N), BF16)
    dp("sel4", (4, 40), BF16)
    dp("wzs_g", (E * 128, OG), BF16)
    dp("wzs_u", (E * 128, OU), BF16)
    dp("wx_g", (40, OG), BF16)
    dp("wx_u", (40, OU), BF16)
    dp("bpool_g", (E, OG), BF16)
    dp("bpool_u", (E, OU), BF16)
    for br in ("g", "u"):
        dp(f"fc1w_{br}", (C, 16), BF16)
        dp(f"fc2w_{br}", (16, 2), BF16)
        dp(f"fc3w_{br}", (2, E), BF16)
        dp(f"fcb1_{br}", (16, 1))
        dp(f"fcb2_{br}", (2, 1))
        dp(f"fcb3_{br}", (E, 1))
    dp("ident", (128, 128))
    dp("identB64", (128, 64), BF16)
    dp("out", (BL, OU, N), BF16, out=True)
    with tile.TileContext(nc) as tc:
        with ExitStack() as ctx:
            _build_body(tc, ctx, nc, P)
    nc.finalize()
    return nc


_NC_CACHE = {}


def _get_nc():
    if "nc" not in _NC_CACHE:
        _NC_CACHE["nc"] = build_nc()
    return _NC_CACHE["nc"]


def _make_in_maps(inputs):
    f32 = lambda a: np.ascontiguousarray(a, dtype=np.float32)
    bf = lambda a: np.ascontiguousarray(np.asarray(a, dtype=np.float32).astype(BF16_NP))
    x = f32(inputs["x"])
    state = f32(inputs["state"])
    emb = f32(inputs["node_embeddings"])
    time, day = f32(inputs["time"]), f32(inputs["day"])
    speed, occupy = f32(inputs["speed"]), f32(inputs["occupy"])

    embT = emb.T                                      # (E, N)
    embSS = np.repeat(embT[:, None, :], 128, axis=1).reshape(E * 128, N)
    embX40 = np.repeat(embT[:, None, :], 4, axis=1).reshape(E * 4, N)
    sel4 = np.tile(np.eye(4, dtype=np.float32), (1, E))  # (4, 40)
    perm_feat = list(range(DIN, C)) + [0, 1]          # state-first

    def pack_w(wpool, operm):
        # wpool (E, K, C, O) -> WZS (E*128, O): per e rows = [k=1 state; k=0 state]
        wp = wpool[..., operm]
        wzs = np.concatenate([wp[:, 1, DIN:, :], wp[:, 0, DIN:, :]], axis=1)
        wzs = wzs.reshape(E * 128, -1)
        # WX (40, O): per e rows = [k1x0, k1x1, k0x0, k0x1]
        wxp = np.stack([wp[:, 1, 0, :], wp[:, 1, 1, :],
                        wp[:, 0, 0, :], wp[:, 0, 1, :]], axis=1)
        wxp = wxp.reshape(E * 4, -1)
        return wzs, wxp

    operm_g = list(range(DOUT, OG)) + list(range(DOUT))  # [r; z]
    wzs_g, wx_g = pack_w(inputs["gate_wpool"], operm_g)
    wzs_u, wx_u = pack_w(inputs["update_wpool"], list(range(OU)))

    identB64 = np.zeros((128, 64), np.float32)
    identB64[64:128, :] = np.eye(64, dtype=np.float32)

    shared = {
        "embT": bf(embT),
        "embSS": bf(embSS),
        "embX40": bf(embX40),
        "sel4": bf(sel4),
        "wzs_g": bf(wzs_g),
        "wzs_u": bf(wzs_u),
        "wx_g": bf(wx_g),
        "wx_u": bf(wx_u),
        "bpool_g": bf(inputs["gate_bpool"][:, operm_g]),
        "bpool_u": bf(inputs["update_bpool"]),
        "ident": np.eye(128, dtype=np.float32),
        "identB64": bf(identB64),
    }
    for br, pre in (("g", "gate"), ("u", "update")):
        shared[f"fc1w_{br}"] = bf(inputs[f"{pre}_fc1_w"][perm_feat, :])
        shared[f"fc2w_{br}"] = bf(inputs[f"{pre}_fc2_w"])
        shared[f"fc3w_{br}"] = bf(inputs[f"{pre}_fc3_w"])
        shared[f"fcb1_{br}"] = f32(inputs[f"{pre}_fc1_b"].reshape(16, 1))
        shared[f"fcb2_{br}"] = f32(inputs[f"{pre}_fc2_b"].reshape(2, 1))
        shared[f"fcb3_{br}"] = f32(inputs[f"{pre}_fc3_b"].reshape(E, 1))

    in_maps = []
    for c in range(NCORES):
        sl = slice(c * BL, (c + 1) * BL)
        m = dict(shared)
        xs, ss = x[sl], state[sl]
        m["x_nat"] = bf(xs)
        m["state_nat"] = bf(ss)
        m["csT"] = bf(np.concatenate([ss.transpose(0, 2, 1),
                                      xs.transpose(0, 2, 1)], axis=1))
        m["stateT"] = bf(ss.transpose(0, 2, 1))
        m["xT"] = bf(xs.transpose(0, 2, 1))
        m["tT"] = bf(time[sl].transpose(0, 2, 1))
        m["dT"] = bf(day[sl].transpose(0, 2, 1))
        m["sT"] = bf(speed[sl].transpose(0, 2, 1))
        m["oT"] = bf(occupy[sl].transpose(0, 2, 1))
        in_maps.append(m)
    return in_maps


def _run(inputs, trace=False):
    nc = _get_nc()
    in_maps = _make_in_maps(inputs)
    res = run_bass_kernel_spmd(nc, in_maps, core_ids=list(range(NCORES)), trace=trace)
    out = np.concatenate(
        [np.asarray(res.results[i]["out"]).astype(np.float32).transpose(0, 2, 1)
         for i in range(NCORES)],
        axis=0,
    )
    return np.ascontiguousarray(out), res


def kernel(**inputs):
    out, _ = _run(inputs, trace=False)
    return out



# revision 3
# speedup vs baseline: 1.1511x; 1.1511x over previous
"""DDGCRN cell on 8 TRN2 NeuronCores — data-parallel over batch.

v2 redesign vs baseline: the DVE and GPSIMD share an SBUF read port, so the
baseline's DVE/gpsimd elementwise split made every concurrent pair of big
tensor_tensor ops run ~3.5x slow. All wide elementwise work now lives on the
DVE (bf16 SBUF TTs hit the 2x_1p perf mode); gpsimd only does the dB
partition-broadcast. Fusions cut whole passes:
  - xp = d*xg (natural) via tensor_scalar with per-partition AP scalar
    (was 7 Act ACTIVATEs per instance).
  - update's cand-natural build (cn) is gone: xp_u = (z_nat*xs)*d in ONE
    scalar_tensor_tensor per tile.
  - filt bias + Mb multiply fused: vpre = (h3p + b3) * Mb in one STT from
    PSUM (filt tile gone).
  - dB broadcast via gpsimd partition_broadcast (was ones-outer PE matmul
    + Act copy).
  - CU rows 0..63 are a copy of S2u rows 64..127 (was computed twice).
  - epilogue on DVE (was gpsimd).

Pipeline (per step s): P1(s+1) hypernet layers interleaved with P2(s)
A-chunks | P6(s-2) | P4(s-1) + P5(s-1) | P3(s) | glue(s-2). Math per
instance:
  filt = hypernet MLP (transposed-feature layout, bf16)
  V = tanh(emb*time*day*speed*occupy*filt)      (10, 883)
  A = relu(V V^T) (883,883 symmetric) + fused row-sums (ACT accum_out)
  d = rsqrt(rowsum) via fast-inverse-sqrt + 1 Newton step (DVE only)
  y^T = (d*xs)^T A  (A symmetric); yd = y^T * dB; Lx = x0 - yd
  out^T = bp.T@embT + WX.T@zx40 + sum_e WZS_e.T @ (embSS_e * S2)  (PSUM acc)

All matmuls bf16 (PSUM f32); inputs pre-cast/pre-transposed on host (pure
layout/dtype prep). Output written transposed bf16, un-transposed on host.
"""

import sys, os

sys.path.insert(0, "/opt/trn_rl_repo")

import numpy as np
import ml_dtypes
from contextlib import ExitStack

import concourse.bass as bass
import concourse.bacc as bacc
import concourse.mybir as mybir
from concourse import tile
from concourse.alu_op_type import AluOpType
from concourse.bass_utils import run_bass_kernel_spmd

AF = mybir.ActivationFunctionType
F32 = mybir.dt.float32
BF16 = mybir.dt.bfloat16
I32 = mybir.dt.int32
BF16_NP = ml_dtypes.bfloat16

B, N, DIN, DOUT, E, CHEB = 64, 883, 2, 64, 10, 2
C = DIN + DOUT  # 66
NCORES = 8
BL = B // NCORES  # 8 batches per core
NT = (N + 127) // 128  # 7 row tiles
OG, OU = 2 * DOUT, DOUT  # 128, 64
SPLITS = [(0, 512), (512, N - 512)]
RSQRT_MAGIC = 0x5F3759DF
MUL = AluOpType.mult
ADD = AluOpType.add

# instance schedule: update(b) >= 4 slots after gate(b)
SEQ = [("g", 0), ("g", 1), ("g", 2), ("g", 3), ("u", 0), ("g", 4), ("u", 1),
       ("g", 5), ("u", 2), ("g", 6), ("u", 3), ("g", 7), ("u", 4), ("u", 5),
       ("u", 6), ("u", 7)]


def _pt(nt):
    return min(128, N - nt * 128)


def _build_body(tc, ctx, nc, P):
    def pool(name, bufs, space="SBUF"):
        return ctx.enter_context(tc.tile_pool(name=name, bufs=bufs, space=space))

    wp = pool("wp", 1)        # static weights
    dat = pool("dat", 2)      # per-batch DMA loads
    act = pool("act", 2)      # per-instance intermediates
    arp = pool("arp", 15)     # relu(A) tiles: 2 instances x 7 in flight
    xnp = pool("xnp", 35)     # natural xs tiles (7/batch, up to 5 batches live)
    znp = pool("znp", 28)     # natural z tiles (7/gate, up to 4 batches live)
    xpp = pool("xpp", 15)     # d*xg natural tiles
    zp = pool("zp", 10)       # packed Z tiles (128, N)
    dnp = pool("dnp", 4)      # rowsum/d helpers
    psp = pool("psp", 2, space="PSUM")  # op + yT accumulators (2-bank tiles)
    psa = pool("psa", 2, space="PSUM")  # A / hypernet / misc (2-bank tiles)

    # ---------------- static setup ----------------
    ident_f = wp.tile([128, 128], F32, tag="identf", name="ident_f")
    nc.sync.dma_start(ident_f[:, :], P["ident"][:, :])
    identB64 = wp.tile([128, 64], BF16, tag="identb64", name="identB64")
    nc.sync.dma_start(identB64[:, :], P["identB64"][:, :])

    def load_bf(pname, shape, tag):
        t = wp.tile(list(shape), BF16, tag=tag, name=pname + "_t")
        nc.sync.dma_start(t[:, :], P[pname][:, :])
        return t

    embT = load_bf("embT", (E, N), "embT")
    embX40 = load_bf("embX40", (40, N), "embX40")
    sel4 = load_bf("sel4", (4, 40), "sel4")
    embSS = []
    for e in range(E):
        t = wp.tile([128, N], BF16, tag=f"embSS{e}", name=f"embSS{e}")
        nc.sync.dma_start(t[:, :], P["embSS"][e * 128:(e + 1) * 128, :])
        embSS.append(t)
    wzs = {}
    for br, On in (("g", OG), ("u", OU)):
        tiles = []
        for e in range(E):
            t = wp.tile([128, On], BF16, tag=f"wzs{br}{e}", name=f"wzs{br}{e}")
            nc.sync.dma_start(t[:, :], P[f"wzs_{br}"][e * 128:(e + 1) * 128, :])
            tiles.append(t)
        wzs[br] = tiles
    wx = {"g": load_bf("wx_g", (40, OG), "wxg"),
          "u": load_bf("wx_u", (40, OU), "wxu")}
    bp = {"g": load_bf("bpool_g", (E, OG), "bpg"),
          "u": load_bf("bpool_u", (E, OU), "bpu")}
    fc = {}
    for br in ("g", "u"):
        fc[("w1", br)] = load_bf(f"fc1w_{br}", (C, 16), f"fc1w{br}")
        fc[("w2", br)] = load_bf(f"fc2w_{br}", (16, 2), f"fc2w{br}")
        fc[("w3", br)] = load_bf(f"fc3w_{br}", (2, E), f"fc3w{br}")
        for nm, shape in (("b1", (16, 1)), ("b2", (2, 1)), ("b3", (E, 1))):
            t = wp.tile(list(shape), F32, tag=f"fc{nm}{br}", name=f"fc{nm}{br}")
            nc.sync.dma_start(t[:, :], P[f"fc{nm}_{br}"][:, :])
            fc[(nm, br)] = t

    # ---------------- per-instance state ----------------
    ST = {}   # (br,b) -> dict of tiles
    BAT = {}  # b -> dict of per-batch tiles

    def batch_load(b):
        """DMA this batch's inputs; build Mb (all on DVE)."""
        d = {}
        xs_nat = []
        for nt in range(NT):
            p = _pt(nt)
            t = xnp.tile([128, C], BF16, tag="xsn", name=f"xsn{b}{nt}")
            nc.sync.dma_start(t[:p, 0:DOUT], P["state_nat"][b, nt * 128:nt * 128 + p, :])
            nc.sync.dma_start(t[:p, DOUT:C], P["x_nat"][b, nt * 128:nt * 128 + p, :])
            xs_nat.append(t)
        d["xs_nat"] = xs_nat
        # csT: rows 0..63 state^T, rows 64..65 x^T
        csT = act.tile([C, N], BF16, tag="csT", name=f"csT{b}", bufs=6)
        nc.sync.dma_start(csT[:, :], P["csT"][b, :, :])
        d["csT"] = csT
        # S2g: rows 64..127 = state^T (k=0 block); rows 0..63 filled in P5(g)
        S2g = act.tile([128, N], BF16, tag="S2g", name=f"S2g{b}", bufs=4)
        nc.sync.dma_start(S2g[64:128, :], P["stateT"][b, :, :])
        d["S2g"] = S2g
        tdso = []
        for nm in ("tT", "dT", "sT", "oT"):
            t = dat.tile([E, N], BF16, tag=nm, name=f"{nm}{b}", bufs=2)
            nc.sync.dma_start(t[:, :], P[nm][b, :, :])
            tdso.append(t)
        p1 = act.tile([E, N], BF16, tag="p1", name=f"p1_{b}", bufs=2)
        nc.vector.tensor_mul(p1[:, :], tdso[0][:, :], tdso[1][:, :])
        p2 = act.tile([E, N], BF16, tag="p2", name=f"p2_{b}", bufs=2)
        nc.vector.tensor_mul(p2[:, :], tdso[2][:, :], tdso[3][:, :])
        p3 = act.tile([E, N], BF16, tag="p1", name=f"p3_{b}", bufs=2)
        nc.vector.tensor_mul(p3[:, :], p1[:, :], p2[:, :])
        Mb = act.tile([E, N], BF16, tag="Mb", name=f"Mb{b}", bufs=5)
        nc.vector.tensor_mul(Mb[:, :], p3[:, :], embT[:, :])
        d["Mb"] = Mb
        BAT[b] = d

    def P1_load(inst):
        br, b = inst
        if br == "g":
            batch_load(b)
            st = ST[inst] = {}
            st["x0T"] = BAT[b]["csT"]
        else:
            st = ST[inst]  # created by glue(gate): has x0T=CU
        rs = dnp.tile([128, 8], F32, tag="rs0", name=f"rs0{br}{b}")
        nc.vector.memset(rs[:, :], 1.0)
        st["rs"] = rs

    def P1_l1(inst):
        br, b = inst
        st = ST[inst]
        xg2 = st["x0T"]
        h1p = psa.tile([16, 896], F32, tag="psB", name=f"h1p{br}{b}")
        h1 = act.tile([16, N], BF16, tag="h1", name=f"h1{br}{b}")
        for s0, sl in SPLITS:
            nc.tensor.matmul(h1p[:16, s0:s0 + sl], fc[("w1", br)][:, :],
                             xg2[:, s0:s0 + sl], start=True, stop=True)
        nc.scalar.activation(h1[:, :], h1p[:16, 0:N],
                             AF.Sigmoid, bias=fc[("b1", br)][:, :])
        st["h1"] = h1

    def P1_l2(inst):
        br, b = inst
        st = ST[inst]
        h2p = psa.tile([2, 896], F32, tag="psB", name=f"h2p{br}{b}")
        h2 = act.tile([2, N], BF16, tag="h2", name=f"h2{br}{b}")
        for s0, sl in SPLITS:
            nc.tensor.matmul(h2p[:2, s0:s0 + sl], fc[("w2", br)][:, :],
                             st["h1"][:, s0:s0 + sl], start=True, stop=True)
        nc.scalar.activation(h2[:, :], h2p[:2, 0:N],
                             AF.Sigmoid, bias=fc[("b2", br)][:, :])
        st["h2"] = h2

    def P1_l3V(inst):
        br, b = inst
        st = ST[inst]
        h3p = psa.tile([E, 896], F32, tag="psB", name=f"h3p{br}{b}")
        for s0, sl in SPLITS:
            nc.tensor.matmul(h3p[:E, s0:s0 + sl], fc[("w3", br)][:, :],
                             st["h2"][:, s0:s0 + sl], start=True, stop=True)
        # vpre = (h3p + b3) * Mb in one fused STT (PSUM src)
        vpre = act.tile([E, N], BF16, tag="vpre", name=f"vpre{br}{b}")
        nc.vector.scalar_tensor_tensor(vpre[:, :], h3p[:E, 0:N],
                                       fc[("b3", br)][:, :], BAT[b]["Mb"][:, :],
                                       ADD, MUL)
        V = act.tile([E, N], BF16, tag="V", name=f"V{br}{b}")
        nc.scalar.activation(V[:, :], vpre[:, :], AF.Tanh)
        st["V"] = V

    def P2_chunk(inst, kts):
        """A = relu(V V^T) + fused row-sums, for a subset of row tiles."""
        br, b = inst
        st = ST[inst]
        V, rsh = st["V"], st["rs"]
        ar = st.setdefault("ar", [])
        for kt in kts:
            p = _pt(kt)
            aps = psa.tile([128, 896], F32, tag="psB", name=f"aps{br}{b}{kt}")
            art = arp.tile([128, N], BF16, tag="ar", name=f"ar{br}{b}{kt}")
            for s0, sl in SPLITS:
                nc.tensor.matmul(aps[:p, s0:s0 + sl],
                                 V[:, kt * 128:kt * 128 + p],
                                 V[:, s0:s0 + sl], start=True, stop=True)
            nc.scalar.activation(art[:p, 0:N], aps[:p, 0:N],
                                 AF.Relu, accum_out=rsh[:p, kt:kt + 1])
            ar.append(art)

    def P3(inst):
        """d = rsqrt(rowsums) on DVE; d-row via PE transpose + DMA; dB via
        gpsimd partition-broadcast; xp = d*xg natural via per-partition TS."""
        br, b = inst
        st = ST[inst]
        rsall = st["rs"]
        tsh = dnp.tile([128, 8], F32, tag="tsh", name=f"tsh{br}{b}")
        nc.vector.tensor_scalar(tsh[:, :].bitcast(I32), rsall[:, :].bitcast(I32),
                                1, None, AluOpType.logical_shift_right)
        tnot = dnp.tile([128, 8], F32, tag="tnot", name=f"tnot{br}{b}")
        nc.vector.tensor_scalar(tnot[:, :].bitcast(I32), tsh[:, :].bitcast(I32),
                                -1, None, AluOpType.bitwise_xor)
        d0 = dnp.tile([128, 8], F32, tag="d0", name=f"d0{br}{b}")
        nc.vector.tensor_scalar(d0[:, :].bitcast(I32), tnot[:, :].bitcast(I32),
                                RSQRT_MAGIC + 1, None, AluOpType.add)
        sq = dnp.tile([128, 8], F32, tag="sq", name=f"sq{br}{b}")
        nc.vector.tensor_mul(sq[:, :], d0[:, :], d0[:, :])
        hx = dnp.tile([128, 8], F32, tag="hx", name=f"hx{br}{b}")
        nc.vector.tensor_mul(hx[:, :], sq[:, :], rsall[:, :])
        cf = dnp.tile([128, 8], F32, tag="cf", name=f"cf{br}{b}")
        nc.vector.tensor_scalar(cf[:, :], hx[:, :], -0.5, 1.5,
                                AluOpType.mult, AluOpType.add)
        dcat = dnp.tile([128, 8], F32, tag="dcat", name=f"dcat{br}{b}")
        nc.vector.tensor_mul(dcat[:, :], d0[:, :], cf[:, :])
        st["dcat"] = dcat
        # d-row: transpose -> flatten DMA -> partition-broadcast to dB
        tp = psa.tile([128, 128], F32, tag="psB", name=f"dtp{br}{b}")
        nc.tensor.transpose(tp[:8, :128], dcat[:, :], ident_f[:, :])
        drs = act.tile([8, 128], BF16, tag="drs", name=f"drs{br}{b}")
        nc.vector.tensor_copy(drs[:, :], tp[:8, :128])
        drow = act.tile([1, 1024], BF16, tag="drow", name=f"drow{br}{b}")
        nc.sync.dma_start(drow[0:1, :], drs[0:8, :])
        dB = act.tile([C, N], BF16, tag="dB", name=f"dB{br}{b}", bufs=3)
        nc.gpsimd.partition_broadcast(dB[:, :], drow[0:1, 0:N], channels=C)
        st["dB"] = dB
        # X4 tile (rows 2..3 = x^T via DMA; rows 0..1 = Lx-x, filled in P5)
        x4 = act.tile([4, 896], BF16, tag="x4", name=f"x4{br}{b}", bufs=4)
        nc.sync.dma_start(x4[2:4, 0:N], P["xT"][b, :, :])
        st["x4"] = x4
        # xp = d * xg in natural layout (per-partition scalar on DVE)
        xs_nat = BAT[b]["xs_nat"]
        xp = []
        for kt in range(NT):
            p = _pt(kt)
            xpt = xpp.tile([128, C], BF16, tag="xp", name=f"xp{br}{b}{kt}")
            if br == "g":
                nc.vector.tensor_scalar(xpt[:p, :], xs_nat[kt][:p, :],
                                        dcat[:p, kt:kt + 1], None, MUL)
            else:
                # cand = [z*state; x]; fold the cn build into the scaling
                zn = ST[("g", b)]["zn"]
                nc.vector.scalar_tensor_tensor(xpt[:p, 0:DOUT],
                                               zn[kt][:p, :],
                                               dcat[:p, kt:kt + 1],
                                               xs_nat[kt][:p, 0:DOUT],
                                               MUL, MUL)
                nc.vector.tensor_scalar(xpt[:p, DOUT:C],
                                        xs_nat[kt][:p, DOUT:C],
                                        dcat[:p, kt:kt + 1], None, MUL)
            xp.append(xpt)
        st["xp"] = xp

    def P4(inst):
        """y^T matmuls + yd = y^T * dB."""
        br, b = inst
        st = ST[inst]
        yt = psp.tile([C, 896], F32, tag="psA", name=f"yt{br}{b}")
        ar, xp = st["ar"], st["xp"]
        for kt in range(NT):
            p = _pt(kt)
            for s0, sl in SPLITS:
                nc.tensor.matmul(yt[:C, s0:s0 + sl], xp[kt][:p, :],
                                 ar[kt][:p, s0:s0 + sl],
                                 start=(kt == 0), stop=(kt == NT - 1))
        dB = st["dB"]
        yd = act.tile([C, N], BF16, tag="yd", name=f"yd{br}{b}")
        nc.vector.tensor_mul(yd[:, :], yt[:C, 0:N], dB[:, :])
        st["yd"] = yd

    def P5(inst):
        """Lx pieces + packed moving operands (all DVE)."""
        br, b = inst
        st = ST[inst]
        yd = st["yd"]
        x0T = st["x0T"]                      # csT (gate) / CU (update)
        S2 = BAT[b]["S2g"] if br == "g" else st["S2u"]
        nc.vector.tensor_sub(S2[0:64, :], x0T[0:64, :], yd[0:64, :])
        x4 = st["x4"]
        nc.vector.tensor_sub(x4[0:2, 0:N], x0T[64:66, :], yd[64:66, :])
        # xrep = SEL4^T X4 (40, sl) psum; zx40 = embX40 * xrep
        zx = act.tile([40, 896], BF16, tag="zx", name=f"zx{br}{b}", bufs=3)
        xrp = psa.tile([40, 896], F32, tag="psB", name=f"xrp{br}{b}")
        for s0, sl in SPLITS:
            nc.tensor.matmul(xrp[:40, s0:s0 + sl], sel4[:, :],
                             x4[0:4, s0:s0 + sl], start=True, stop=True)
        nc.vector.tensor_mul(zx[:, 0:N], embX40[:, :], xrp[:40, 0:N])
        st["zx"] = zx
        # packed Z moving tiles, all on DVE (bf16 SBUF TT = 2x mode)
        zt = []
        for e in range(E):
            z = zp.tile([128, N], BF16, tag="Z", name=f"Z{br}{b}{e}", bufs=10)
            nc.vector.tensor_mul(z[:, :], embSS[e][:, :], S2[:, :])
            zt.append(z)
        st["zt"] = zt

    def P6(inst):
        """Final per-node einsum (packed contraction) + output activation."""
        br, b = inst
        st = ST[inst]
        On = OG if br == "g" else OU
        outf = AF.Sigmoid if br == "g" else AF.Tanh
        op = psp.tile([On, 896], F32, tag="psA", name=f"op{br}{b}")
        zx, zt = st["zx"], st["zt"]
        for s0, sl in SPLITS:
            nc.tensor.matmul(op[:On, s0:s0 + sl], bp[br][:, :],
                             embT[:, s0:s0 + sl], start=True, stop=False)
            nc.tensor.matmul(op[:On, s0:s0 + sl], wx[br][:, :],
                             zx[:, s0:s0 + sl], start=False, stop=False)
            for e in range(E):
                nc.tensor.matmul(op[:On, s0:s0 + sl], wzs[br][e][:, :],
                                 zt[e][:, s0:s0 + sl], start=False,
                                 stop=(e == E - 1))
        zout = act.tile([On, N], BF16, tag=f"zout{br}",
                        name=f"zout{br}{b}", bufs=(5 if br == "g" else 2))
        nc.scalar.activation(zout[:, :], op[:On, 0:N], outf)
        st["zout"] = zout

    def glue(inst):
        """After P6: gate -> build update inputs; update -> epilogue + store."""
        br, b = inst
        if br == "g":
            zr = ST[inst]["zout"]  # (128, N): rows 0..63 = r, 64..127 = z
            S2g = BAT[b]["S2g"]    # rows 64..127 = state^T
            csT = BAT[b]["csT"]
            ust = {}
            ST[("u", b)] = ust
            # S2u rows 64..127 = cand-state = z * state (k=0 block)
            S2u = act.tile([128, N], BF16, tag="S2u", name=f"S2u{b}", bufs=3)
            nc.vector.tensor_mul(S2u[64:128, :], zr[64:128, :], S2g[64:128, :])
            ust["S2u"] = S2u
            # CU: update hypernet input; rows 0..63 copy of S2u rows 64..127
            CU = act.tile([C, N], BF16, tag="CU", name=f"CU{b}", bufs=3)
            nc.vector.tensor_copy(CU[0:64, :], S2u[64:128, :])
            nc.vector.tensor_copy(CU[64:66, :], csT[64:66, :])
            ust["x0T"] = CU
            # natural z tiles for update's fused xp (PE transpose + DVE copy)
            zn_l = []
            for nt in range(NT):
                p = _pt(nt)
                zps = psa.tile([128, 64], BF16, tag="psB", name=f"znp{b}{nt}")
                nc.tensor.transpose(zps[:p, :DOUT],
                                    zr[64:128, nt * 128:nt * 128 + p],
                                    identB64[64:128, :])
                zn = znp.tile([128, DOUT], BF16, tag="zn", name=f"zn{b}{nt}")
                nc.vector.tensor_copy(zn[:p, :], zps[:p, :DOUT])
                zn_l.append(zn)
            ST[inst]["zn"] = zn_l
        else:
            # epilogue on DVE; result only feeds the output DMA
            hc = ST[inst]["zout"]          # (64, N) at base 0
            r = ST[("g", b)]["zout"]       # gate zout rows 0..63 = r
            csT = BAT[b]["csT"]            # rows 0..63 = state^T (base 0)
            t1 = act.tile([OU, N], BF16, tag="t1", name=f"t1_{b}", bufs=2)
            nc.vector.tensor_sub(t1[:, :], csT[0:64, :], hc[:, :])
            t2 = act.tile([OU, N], BF16, tag="t2", name=f"t2_{b}", bufs=2)
            nc.vector.tensor_mul(t2[:, :], r[0:64, :], t1[:, :])
            outT = act.tile([OU, N], BF16, tag="outT", name=f"outT{b}")
            nc.vector.tensor_add(outT[:, :], t2[:, :], hc[:, :])
            nc.sync.dma_start(P["out"][b, :, :], outT[:, :])

    # ---------------- pipeline driver ----------------
    M = len(SEQ)
    P1_load(SEQ[0]); P1_l1(SEQ[0]); P1_l2(SEQ[0]); P1_l3V(SEQ[0])
    for s in range(M + 2):
        nxt = SEQ[s + 1] if s + 1 < M else None
        cur = SEQ[s] if s < M else None
        if nxt:
            P1_load(nxt)
            P1_l1(nxt)
        if cur:
            P2_chunk(cur, range(0, 4))
        if nxt:
            P1_l2(nxt)
        if cur:
            P2_chunk(cur, range(4, 7))
        if nxt:
            P1_l3V(nxt)
        if 0 <= s - 2 < M:
            P6(SEQ[s - 2])
        if 0 <= s - 1 < M:
            P4(SEQ[s - 1])
            P5(SEQ[s - 1])
        if cur:
            P3(cur)
        if 0 <= s - 2 < M:
            glue(SEQ[s - 2])


def build_nc():
    nc = bacc.Bacc()
    P = {}

    def dp(name, shape, dtype=F32, out=False):
        P[name] = nc.declare_dram_parameter(name, list(shape), dtype, isOutput=out)

    dp("x_nat", (BL, N, DIN), BF16)
    dp("state_nat", (BL, N, DOUT), BF16)
    dp("csT", (BL, C, N), BF16)
    dp("stateT", (BL, DOUT, N), BF16)
    dp("xT", (BL, DIN, N), BF16)
    for nm in ("tT", "dT", "sT", "oT"):
        dp(nm, (BL, E, N), BF16)
    dp("embT", (E, N), BF16)
    dp("embSS", (E * 128, N), BF16)
    dp("embX40", (40, N), BF16)
    dp("sel4", (4, 40), BF16)
    dp("wzs_g", (E * 128, OG), BF16)
    dp("wzs_u", (E * 128, OU), BF16)
    dp("wx_g", (40, OG), BF16)
    dp("wx_u", (40, OU), BF16)
    dp("bpool_g", (E, OG), BF16)
    dp("bpool_u", (E, OU), BF16)
    for br in ("g", "u"):
        dp(f"fc1w_{br}", (C, 16), BF16)
        dp(f"fc2w_{br}", (16, 2), BF16)
        dp(f"fc3w_{br}", (2, E), BF16)
        dp(f"fcb1_{br}", (16, 1))
        dp(f"fcb2_{br}", (2, 1))
        dp(f"fcb3_{br}", (E, 1))
    dp("ident", (128, 128))
    dp("identB64", (128, 64), BF16)
    dp("out", (BL, OU, N), BF16, out=True)
    with tile.TileContext(nc) as tc:
        with ExitStack() as ctx:
            _build_body(tc, ctx, nc, P)
    nc.finalize()
    return nc


_NC_CACHE = {}


def _get_nc():
    if "nc" not in _NC_CACHE:
        _NC_CACHE["nc"] = build_nc()
    return _NC_CACHE["nc"]


def _make_in_maps(inputs):
    f32 = lambda a: np.ascontiguousarray(a, dtype=np.float32)
    bf = lambda a: np.ascontiguousarray(np.asarray(a, dtype=np.float32).astype(BF16_NP))
    x = f32(inputs["x"])
    state = f32(inputs["state"])
    emb = f32(inputs["node_embeddings"])
    time, day = f32(inputs["time"]), f32(inputs["day"])
    speed, occupy = f32(inputs["speed"]), f32(inputs["occupy"])

    embT = emb.T                                      # (E, N)
    embSS = np.repeat(embT[:, None, :], 128, axis=1).reshape(E * 128, N)
    embX40 = np.repeat(embT[:, None, :], 4, axis=1).reshape(E * 4, N)
    sel4 = np.tile(np.eye(4, dtype=np.float32), (1, E))  # (4, 40)
    perm_feat = list(range(DIN, C)) + [0, 1]          # state-first

    def pack_w(wpool, operm):
        # wpool (E, K, C, O) -> WZS (E*128, O): per e rows = [k=1 state; k=0 state]
        wp = wpool[..., operm]
        wzs = np.concatenate([wp[:, 1, DIN:, :], wp[:, 0, DIN:, :]], axis=1)
        wzs = wzs.reshape(E * 128, -1)
        # WX (40, O): per e rows = [k1x0, k1x1, k0x0, k0x1]
        wxp = np.stack([wp[:, 1, 0, :], wp[:, 1, 1, :],
                        wp[:, 0, 0, :], wp[:, 0, 1, :]], axis=1)
        wxp = wxp.reshape(E * 4, -1)
        return wzs, wxp

    operm_g = list(range(DOUT, OG)) + list(range(DOUT))  # [r; z]
    wzs_g, wx_g = pack_w(inputs["gate_wpool"], operm_g)
    wzs_u, wx_u = pack_w(inputs["update_wpool"], list(range(OU)))

    identB64 = np.zeros((128, 64), np.float32)
    identB64[64:128, :] = np.eye(64, dtype=np.float32)

    shared = {
        "embT": bf(embT),
        "embSS": bf(embSS),
        "embX40": bf(embX40),
        "sel4": bf(sel4),
        "wzs_g": bf(wzs_g),
        "wzs_u": bf(wzs_u),
        "wx_g": bf(wx_g),
        "wx_u": bf(wx_u),
        "bpool_g": bf(inputs["gate_bpool"][:, operm_g]),
        "bpool_u": bf(inputs["update_bpool"]),
        "ident": np.eye(128, dtype=np.float32),
        "identB64": bf(identB64),
    }
    for br, pre in (("g", "gate"), ("u", "update")):
        shared[f"fc1w_{br}"] = bf(inputs[f"{pre}_fc1_w"][perm_feat, :])
        shared[f"fc2w_{br}"] = bf(inputs[f"{pre}_fc2_w"])
        shared[f"fc3w_{br}"] = bf(inputs[f"{pre}_fc3_w"])
        shared[f"fcb1_{br}"] = f32(inputs[f"{pre}_fc1_b"].reshape(16, 1))
        shared[f"fcb2_{br}"] = f32(inputs[f"{pre}_fc2_b"].reshape(2, 1))
        shared[f"fcb3_{br}"] = f32(inputs[f"{pre}_fc3_b"].reshape(E, 1))

    in_maps = []
    for c in range(NCORES):
        sl = slice(c * BL, (c + 1) * BL)
        m = dict(shared)
        xs, ss = x[sl], state[sl]
        m["x_nat"] = bf(xs)
        m["state_nat"] = bf(ss)
        m["csT"] = bf(np.concatenate([ss.transpose(0, 2, 1),
                                      xs.transpose(0, 2, 1)], axis=1))
        m["stateT"] = bf(ss.transpose(0, 2, 1))
        m["xT"] = bf(xs.transpose(0, 2, 1))
        m["tT"] = bf(time[sl].transpose(0, 2, 1))
        m["dT"] = bf(day[sl].transpose(0, 2, 1))
        m["sT"] = bf(speed[sl].transpose(0, 2, 1))
        m["oT"] = bf(occupy[sl].transpose(0, 2, 1))
        in_maps.append(m)
    return in_maps


def _run(inputs, trace=False):
    nc = _get_nc()
    in_maps = _make_in_maps(inputs)
    res = run_bass_kernel_spmd(nc, in_maps, core_ids=list(range(NCORES)), trace=trace)
    out = np.concatenate(
        [np.asarray(res.results[i]["out"]).astype(np.float32).transpose(0, 2, 1)
         for i in range(NCORES)],
        axis=0,
    )
    return np.ascontiguousarray(out), res


def kernel(**inputs):
    out, _ = _run(inputs, trace=False)
    return out


# revision 4
# speedup vs baseline: 1.2246x; 1.0638x over previous
"""DDGCRN cell on 8 TRN2 NeuronCores — data-parallel over batch.

v3: v2 (all wide elementwise on DVE; gpsimd off the shared-port hot path;
STT/TS fusions for xp, cand-natural, filt-bias) plus:
  - packed input DMAs: one DMA per batch for the natural xs tiles
    (128 x 7*68) and one for time/day/speed/occupy (10 x 4*896) — the
    Sync queue was half-busy just issuing 22 descriptors per batch.
  - dB built by a stride-0-broadcast DMA from the flattened d-row
    (replaces gpsimd partition_broadcast, which contended with the DVE
    for the shared SBUF port).
  - dependency-ordered DMA queues: loads on the sync queue; stores and
    the d-row flatten/broadcast (which wait on DVE results) on the
    gpsimd queue so they can't head-of-line-block input loads.
  - step reorder: P6/zout and the gate z-transposes run at the step head
    (they only need last step's results), so the PE never waits on the
    yd->subs DVE chain; P5 computes the x4 rows first to unblock xrep.

Math per instance:
  filt = hypernet MLP (transposed-feature layout, bf16)
  V = tanh(emb*time*day*speed*occupy*filt)      (10, 883)
  A = relu(V V^T) (883,883 symmetric) + fused row-sums (ACT accum_out)
  d = rsqrt(rowsum) via fast-inverse-sqrt + 1 Newton step (DVE only)
  y^T = (d*xs)^T A  (A symmetric); yd = y^T * dB; Lx = x0 - yd
  out^T = bp.T@embT + WX.T@zx40 + sum_e WZS_e.T @ (embSS_e * S2)  (PSUM acc)

All matmuls bf16 (PSUM f32); inputs pre-cast/pre-transposed on host (pure
layout/dtype prep). Output written transposed bf16, un-transposed on host.
"""

import sys, os

sys.path.insert(0, "/opt/trn_rl_repo")

import numpy as np
import ml_dtypes
from contextlib import ExitStack

import concourse.bass as bass
import concourse.bacc as bacc
import concourse.mybir as mybir
from concourse import tile
from concourse.alu_op_type import AluOpType
from concourse.bass_types import AP
from concourse.bass_utils import run_bass_kernel_spmd

AF = mybir.ActivationFunctionType
F32 = mybir.dt.float32
BF16 = mybir.dt.bfloat16
I32 = mybir.dt.int32
BF16_NP = ml_dtypes.bfloat16

B, N, DIN, DOUT, E, CHEB = 64, 883, 2, 64, 10, 2
C = DIN + DOUT  # 66
NCORES = 8
BL = B // NCORES  # 8 batches per core
NT = (N + 127) // 128  # 7 row tiles
OG, OU = 2 * DOUT, DOUT  # 128, 64
SPLITS = [(0, 512), (512, N - 512)]
RSQRT_MAGIC = 0x5F3759DF
MUL = AluOpType.mult
ADD = AluOpType.add
XW = 68            # packed natural-xs tile pitch (64 state + 2 x + pad)
NP = 896           # padded column count

# instance schedule: update(b) >= 4 slots after gate(b)
SEQ = [("g", 0), ("g", 1), ("g", 2), ("g", 3), ("u", 0), ("g", 4), ("u", 1),
       ("g", 5), ("u", 2), ("g", 6), ("u", 3), ("g", 7), ("u", 4), ("u", 5),
       ("u", 6), ("u", 7)]


def _pt(nt):
    return min(128, N - nt * 128)


def _bcast(ap, reps):
    """Repeat a single-partition AP `reps` times via a stride-0 dim."""
    return AP(ap.tensor, ap.offset, [ap.ap[0], [0, reps]] + list(ap.ap[1:]))


def _build_body(tc, ctx, nc, P):
    def pool(name, bufs, space="SBUF"):
        return ctx.enter_context(tc.tile_pool(name=name, bufs=bufs, space=space))

    wp = pool("wp", 1)        # static weights
    dat = pool("dat", 2)      # per-batch DMA loads
    act = pool("act", 2)      # per-instance intermediates
    arp = pool("arp", 15)     # relu(A) tiles: 2 instances x 7 in flight
    xnp = pool("xnp", 5)      # packed natural xs tiles (1/batch, 5 batches live)
    znp = pool("znp", 28)     # natural z tiles (7/gate, up to 4 batches live)
    xpp = pool("xpp", 15)     # d*xg natural tiles
    zp = pool("zp", 10)       # packed Z tiles (128, N)
    dnp = pool("dnp", 4)      # rowsum/d helpers
    psp = pool("psp", 2, space="PSUM")  # op + yT accumulators (2-bank tiles)
    psa = pool("psa", 2, space="PSUM")  # A / hypernet / misc (2-bank tiles)

    # ---------------- static setup ----------------
    ident_f = wp.tile([128, 128], F32, tag="identf", name="ident_f")
    nc.sync.dma_start(ident_f[:, :], P["ident"][:, :])
    identB64 = wp.tile([128, 64], BF16, tag="identb64", name="identB64")
    nc.sync.dma_start(identB64[:, :], P["identB64"][:, :])

    def load_bf(pname, shape, tag):
        t = wp.tile(list(shape), BF16, tag=tag, name=pname + "_t")
        nc.sync.dma_start(t[:, :], P[pname][:, :])
        return t

    embT = load_bf("embT", (E, N), "embT")
    embX40 = load_bf("embX40", (40, N), "embX40")
    sel4 = load_bf("sel4", (4, 40), "sel4")
    embSS = []
    for e in range(E):
        t = wp.tile([128, N], BF16, tag=f"embSS{e}", name=f"embSS{e}")
        nc.sync.dma_start(t[:, :], P["embSS"][e * 128:(e + 1) * 128, :])
        embSS.append(t)
    wzs = {}
    for br, On in (("g", OG), ("u", OU)):
        tiles = []
        for e in range(E):
            t = wp.tile([128, On], BF16, tag=f"wzs{br}{e}", name=f"wzs{br}{e}")
            nc.sync.dma_start(t[:, :], P[f"wzs_{br}"][e * 128:(e + 1) * 128, :])
            tiles.append(t)
        wzs[br] = tiles
    wx = {"g": load_bf("wx_g", (40, OG), "wxg"),
          "u": load_bf("wx_u", (40, OU), "wxu")}
    bp = {"g": load_bf("bpool_g", (E, OG), "bpg"),
          "u": load_bf("bpool_u", (E, OU), "bpu")}
    fc = {}
    for br in ("g", "u"):
        fc[("w1", br)] = load_bf(f"fc1w_{br}", (C, 16), f"fc1w{br}")
        fc[("w2", br)] = load_bf(f"fc2w_{br}", (16, 2), f"fc2w{br}")
        fc[("w3", br)] = load_bf(f"fc3w_{br}", (2, E), f"fc3w{br}")
        for nm, shape in (("b1", (16, 1)), ("b2", (2, 1)), ("b3", (E, 1))):
            t = wp.tile(list(shape), F32, tag=f"fc{nm}{br}", name=f"fc{nm}{br}")
            nc.sync.dma_start(t[:, :], P[f"fc{nm}_{br}"][:, :])
            fc[(nm, br)] = t

    # ---------------- per-instance state ----------------
    ST = {}   # (br,b) -> dict of tiles
    BAT = {}  # b -> dict of per-batch tiles

    def batch_load(b):
        """DMA this batch's inputs (packed); build Mb on DVE."""
        d = {}
        xsn = xnp.tile([128, NT * XW], BF16, tag="xsn", name=f"xsn{b}")
        nc.sync.dma_start(xsn[:, :], P["xsn"][b, :, :])
        d["xsn"] = xsn
        csT = act.tile([C, N], BF16, tag="csT", name=f"csT{b}", bufs=6)
        nc.sync.dma_start(csT[:, :], P["csT"][b, :, :])
        d["csT"] = csT
        S2g = act.tile([128, N], BF16, tag="S2g", name=f"S2g{b}", bufs=4)
        nc.sync.dma_start(S2g[64:128, :], P["stateT"][b, :, :])
        d["S2g"] = S2g
        td4 = dat.tile([E, 4 * NP], BF16, tag="td4", name=f"td4{b}", bufs=2)
        nc.sync.dma_start(td4[:, :], P["tdso4"][b, :, :])
        p1 = act.tile([E, N], BF16, tag="p1", name=f"p1_{b}", bufs=2)
        nc.vector.tensor_mul(p1[:, :], td4[:, 0:N], td4[:, NP:NP + N])
        p2 = act.tile([E, N], BF16, tag="p2", name=f"p2_{b}", bufs=2)
        nc.vector.tensor_mul(p2[:, :], td4[:, 2 * NP:2 * NP + N],
                             td4[:, 3 * NP:3 * NP + N])
        p3 = act.tile([E, N], BF16, tag="p1", name=f"p3_{b}", bufs=2)
        nc.vector.tensor_mul(p3[:, :], p1[:, :], p2[:, :])
        Mb = act.tile([E, N], BF16, tag="Mb", name=f"Mb{b}", bufs=5)
        nc.vector.tensor_mul(Mb[:, :], p3[:, :], embT[:, :])
        d["Mb"] = Mb
        BAT[b] = d

    def P1_load(inst):
        br, b = inst
        if br == "g":
            batch_load(b)
            st = ST[inst] = {}
            st["x0T"] = BAT[b]["csT"]
        else:
            st = ST[inst]  # created by glue(gate): has x0T=CU
        rs = dnp.tile([128, 8], F32, tag="rs0", name=f"rs0{br}{b}")
        nc.vector.memset(rs[:, :], 1.0)
        st["rs"] = rs

    def P1_l1(inst):
        br, b = inst
        st = ST[inst]
        xg2 = st["x0T"]
        h1p = psa.tile([16, 896], F32, tag="psB", name=f"h1p{br}{b}")
        h1 = act.tile([16, N], BF16, tag="h1", name=f"h1{br}{b}")
        for s0, sl in SPLITS:
            nc.tensor.matmul(h1p[:16, s0:s0 + sl], fc[("w1", br)][:, :],
                             xg2[:, s0:s0 + sl], start=True, stop=True)
        nc.scalar.activation(h1[:, :], h1p[:16, 0:N],
                             AF.Sigmoid, bias=fc[("b1", br)][:, :])
        st["h1"] = h1

    def P1_l2(inst):
        br, b = inst
        st = ST[inst]
        h2p = psa.tile([2, 896], F32, tag="psB", name=f"h2p{br}{b}")
        h2 = act.tile([2, N], BF16, tag="h2", name=f"h2{br}{b}")
        for s0, sl in SPLITS:
            nc.tensor.matmul(h2p[:2, s0:s0 + sl], fc[("w2", br)][:, :],
                             st["h1"][:, s0:s0 + sl], start=True, stop=True)
        nc.scalar.activation(h2[:, :], h2p[:2, 0:N],
                             AF.Sigmoid, bias=fc[("b2", br)][:, :])
        st["h2"] = h2

    def P1_l3V(inst):
        br, b = inst
        st = ST[inst]
        h3p = psa.tile([E, 896], F32, tag="psB", name=f"h3p{br}{b}")
        for s0, sl in SPLITS:
            nc.tensor.matmul(h3p[:E, s0:s0 + sl], fc[("w3", br)][:, :],
                             st["h2"][:, s0:s0 + sl], start=True, stop=True)
        # vpre = (h3p + b3) * Mb in one fused STT (PSUM src)
        vpre = act.tile([E, N], BF16, tag="vpre", name=f"vpre{br}{b}")
        nc.vector.scalar_tensor_tensor(vpre[:, :], h3p[:E, 0:N],
                                       fc[("b3", br)][:, :], BAT[b]["Mb"][:, :],
                                       ADD, MUL)
        V = act.tile([E, N], BF16, tag="V", name=f"V{br}{b}")
        nc.scalar.activation(V[:, :], vpre[:, :], AF.Tanh)
        st["V"] = V

    def P2_chunk(inst, kts):
        """A = relu(V V^T) + fused row-sums, for a subset of row tiles."""
        br, b = inst
        st = ST[inst]
        V, rsh = st["V"], st["rs"]
        ar = st.setdefault("ar", [])
        for kt in kts:
            p = _pt(kt)
            aps = psa.tile([128, 896], F32, tag="psB", name=f"aps{br}{b}{kt}")
            art = arp.tile([128, N], BF16, tag="ar", name=f"ar{br}{b}{kt}")
            for s0, sl in SPLITS:
                nc.tensor.matmul(aps[:p, s0:s0 + sl],
                                 V[:, kt * 128:kt * 128 + p],
                                 V[:, s0:s0 + sl], start=True, stop=True)
            nc.scalar.activation(art[:p, 0:N], aps[:p, 0:N],
                                 AF.Relu, accum_out=rsh[:p, kt:kt + 1])
            ar.append(art)

    def P3(inst):
        """d = rsqrt(rowsums) on DVE; dB via flatten + broadcast DMA;
        xp = d*xg natural via per-partition TS/STT on DVE."""
        br, b = inst
        st = ST[inst]
        rsall = st["rs"]
        tsh = dnp.tile([128, 8], F32, tag="tsh", name=f"tsh{br}{b}")
        nc.vector.tensor_scalar(tsh[:, :].bitcast(I32), rsall[:, :].bitcast(I32),
                                1, None, AluOpType.logical_shift_right)
        tnot = dnp.tile([128, 8], F32, tag="tnot", name=f"tnot{br}{b}")
        nc.vector.tensor_scalar(tnot[:, :].bitcast(I32), tsh[:, :].bitcast(I32),
                                -1, None, AluOpType.bitwise_xor)
        d0 = dnp.tile([128, 8], F32, tag="d0", name=f"d0{br}{b}")
        nc.vector.tensor_scalar(d0[:, :].bitcast(I32), tnot[:, :].bitcast(I32),
                                RSQRT_MAGIC + 1, None, AluOpType.add)
        sq = dnp.tile([128, 8], F32, tag="sq", name=f"sq{br}{b}")
        nc.vector.tensor_mul(sq[:, :], d0[:, :], d0[:, :])
        hx = dnp.tile([128, 8], F32, tag="hx", name=f"hx{br}{b}")
        nc.vector.tensor_mul(hx[:, :], sq[:, :], rsall[:, :])
        cf = dnp.tile([128, 8], F32, tag="cf", name=f"cf{br}{b}")
        nc.vector.tensor_scalar(cf[:, :], hx[:, :], -0.5, 1.5,
                                AluOpType.mult, AluOpType.add)
        dcat = dnp.tile([128, 8], F32, tag="dcat", name=f"dcat{br}{b}")
        nc.vector.tensor_mul(dcat[:, :], d0[:, :], cf[:, :])
        st["dcat"] = dcat
        # d-row: PE transpose -> DVE bf16 copy -> flatten DMA -> broadcast DMA
        tp = psa.tile([128, 128], F32, tag="psB", name=f"dtp{br}{b}")
        nc.tensor.transpose(tp[:8, :128], dcat[:, :], ident_f[:, :])
        drs = act.tile([8, 128], BF16, tag="drs", name=f"drs{br}{b}")
        nc.vector.tensor_copy(drs[:, :], tp[:8, :128])
        drow = act.tile([1, 1024], BF16, tag="drow", name=f"drow{br}{b}")
        nc.gpsimd.dma_start(drow[0:1, :], drs[0:8, :])
        dB = act.tile([C, NP], BF16, tag="dB", name=f"dB{br}{b}", bufs=3)
        nc.gpsimd.dma_start(dB[:, :], _bcast(drow[0:1, 0:NP], C))
        st["dB"] = dB
        # X4 tile (rows 2..3 = x^T via DMA; rows 0..1 = Lx-x, filled in P5)
        x4 = act.tile([4, 896], BF16, tag="x4", name=f"x4{br}{b}", bufs=4)
        nc.sync.dma_start(x4[2:4, 0:N], P["xT"][b, :, :])
        st["x4"] = x4
        # xp = d * xg in natural layout (per-partition scalar on DVE)
        xsn = BAT[b]["xsn"]
        xp = []
        for kt in range(NT):
            p = _pt(kt)
            c0 = kt * XW
            xpt = xpp.tile([128, C], BF16, tag="xp", name=f"xp{br}{b}{kt}")
            if br == "g":
                nc.vector.tensor_scalar(xpt[:p, :], xsn[:p, c0:c0 + C],
                                        dcat[:p, kt:kt + 1], None, MUL)
            else:
                # cand = [z*state; x]; fold the cn build into the scaling
                zn = ST[("g", b)]["zn"]
                nc.vector.scalar_tensor_tensor(xpt[:p, 0:DOUT],
                                               zn[kt][:p, :],
                                               dcat[:p, kt:kt + 1],
                                               xsn[:p, c0:c0 + DOUT],
                                               MUL, MUL)
                nc.vector.tensor_scalar(xpt[:p, DOUT:C],
                                        xsn[:p, c0 + DOUT:c0 + C],
                                        dcat[:p, kt:kt + 1], None, MUL)
            xp.append(xpt)
        st["xp"] = xp

    def P4(inst):
        """y^T matmuls + yd = y^T * dB."""
        br, b = inst
        st = ST[inst]
        yt = psp.tile([C, 896], F32, tag="psA", name=f"yt{br}{b}")
        ar, xp = st["ar"], st["xp"]
        for kt in range(NT):
            p = _pt(kt)
            for s0, sl in SPLITS:
                nc.tensor.matmul(yt[:C, s0:s0 + sl], xp[kt][:p, :],
                                 ar[kt][:p, s0:s0 + sl],
                                 start=(kt == 0), stop=(kt == NT - 1))
        dB = st["dB"]
        yd = act.tile([C, N], BF16, tag="yd", name=f"yd{br}{b}")
        nc.vector.tensor_mul(yd[:, :], yt[:C, 0:N], dB[:, 0:N])
        st["yd"] = yd

    def P5(inst):
        """Lx pieces + packed moving operands (all DVE); x4 first so the
        PE's xrep matmul unblocks as early as possible."""
        br, b = inst
        st = ST[inst]
        yd = st["yd"]
        x0T = st["x0T"]                      # csT (gate) / CU (update)
        S2 = BAT[b]["S2g"] if br == "g" else st["S2u"]
        x4 = st["x4"]
        nc.vector.tensor_sub(x4[0:2, 0:N], x0T[64:66, :], yd[64:66, :])
        nc.vector.tensor_sub(S2[0:64, :], x0T[0:64, :], yd[0:64, :])
        # xrep = SEL4^T X4 (40, sl) psum; zx40 = embX40 * xrep
        zx = act.tile([40, 896], BF16, tag="zx", name=f"zx{br}{b}", bufs=3)
        xrp = psa.tile([40, 896], F32, tag="psB", name=f"xrp{br}{b}")
        for s0, sl in SPLITS:
            nc.tensor.matmul(xrp[:40, s0:s0 + sl], sel4[:, :],
                             x4[0:4, s0:s0 + sl], start=True, stop=True)
        nc.vector.tensor_mul(zx[:, 0:N], embX40[:, :], xrp[:40, 0:N])
        st["zx"] = zx
        # packed Z moving tiles, all on DVE (bf16 SBUF TT = 2x mode)
        zt = []
        for e in range(E):
            z = zp.tile([128, N], BF16, tag="Z", name=f"Z{br}{b}{e}", bufs=10)
            nc.vector.tensor_mul(z[:, :], embSS[e][:, :], S2[:, :])
            zt.append(z)
        st["zt"] = zt

    def P6(inst):
        """Final per-node einsum (packed contraction) + output activation."""
        br, b = inst
        st = ST[inst]
        On = OG if br == "g" else OU
        outf = AF.Sigmoid if br == "g" else AF.Tanh
        op = psp.tile([On, 896], F32, tag="psA", name=f"op{br}{b}")
        zx, zt = st["zx"], st["zt"]
        for s0, sl in SPLITS:
            nc.tensor.matmul(op[:On, s0:s0 + sl], bp[br][:, :],
                             embT[:, s0:s0 + sl], start=True, stop=False)
            nc.tensor.matmul(op[:On, s0:s0 + sl], wx[br][:, :],
                             zx[:, s0:s0 + sl], start=False, stop=False)
            for e in range(E):
                nc.tensor.matmul(op[:On, s0:s0 + sl], wzs[br][e][:, :],
                                 zt[e][:, s0:s0 + sl], start=False,
                                 stop=(e == E - 1))
        zout = act.tile([On, N], BF16, tag=f"zout{br}",
                        name=f"zout{br}{b}", bufs=(5 if br == "g" else 2))
        nc.scalar.activation(zout[:, :], op[:On, 0:N], outf)
        st["zout"] = zout

    def glue_pe(inst):
        """Natural z tiles for update's fused xp (PE transpose + DVE copy).
        Runs at the step head: only needs last step's zout."""
        br, b = inst
        if br != "g":
            return
        zr = ST[inst]["zout"]
        zn_l = []
        for nt in range(NT):
            p = _pt(nt)
            zps = psa.tile([128, 64], BF16, tag="psB", name=f"znp{b}{nt}")
            nc.tensor.transpose(zps[:p, :DOUT],
                                zr[64:128, nt * 128:nt * 128 + p],
                                identB64[64:128, :])
            zn = znp.tile([128, DOUT], BF16, tag="zn", name=f"zn{b}{nt}")
            nc.vector.tensor_copy(zn[:p, :], zps[:p, :DOUT])
            zn_l.append(zn)
        ST[inst]["zn"] = zn_l

    def glue_rest(inst):
        br, b = inst
        if br == "g":
            zr = ST[inst]["zout"]  # (128, N): rows 0..63 = r, 64..127 = z
            S2g = BAT[b]["S2g"]    # rows 64..127 = state^T
            csT = BAT[b]["csT"]
            ust = ST[("u", b)] = {}
            # S2u rows 64..127 = cand-state = z * state (k=0 block)
            S2u = act.tile([128, N], BF16, tag="S2u", name=f"S2u{b}", bufs=3)
            nc.vector.tensor_mul(S2u[64:128, :], zr[64:128, :], S2g[64:128, :])
            ust["S2u"] = S2u
            # CU: update hypernet input; rows 0..63 copy of S2u rows 64..127
            CU = act.tile([C, N], BF16, tag="CU", name=f"CU{b}", bufs=3)
            nc.vector.tensor_copy(CU[0:64, :], S2u[64:128, :])
            nc.vector.tensor_copy(CU[64:66, :], csT[64:66, :])
            ust["x0T"] = CU
            ust["zn"] = ST[inst]["zn"]
        else:
            # epilogue on DVE; result only feeds the output DMA
            hc = ST[inst]["zout"]          # (64, N) at base 0
            r = ST[("g", b)]["zout"]       # gate zout rows 0..63 = r
            csT = BAT[b]["csT"]            # rows 0..63 = state^T (base 0)
            t1 = act.tile([OU, N], BF16, tag="t1", name=f"t1_{b}", bufs=2)
            nc.vector.tensor_sub(t1[:, :], csT[0:64, :], hc[:, :])
            t2 = act.tile([OU, N], BF16, tag="t2", name=f"t2_{b}", bufs=2)
            nc.vector.tensor_mul(t2[:, :], r[0:64, :], t1[:, :])
            outT = act.tile([OU, N], BF16, tag="outT", name=f"outT{b}")
            nc.vector.tensor_add(outT[:, :], t2[:, :], hc[:, :])
            nc.gpsimd.dma_start(P["out"][b, :, :], outT[:, :])

    # ---------------- pipeline driver ----------------
    M = len(SEQ)
    P1_load(SEQ[0]); P1_l1(SEQ[0]); P1_l2(SEQ[0]); P1_l3V(SEQ[0])
    for s in range(M + 2):
        nxt = SEQ[s + 1] if s + 1 < M else None
        cur = SEQ[s] if s < M else None
        if nxt:
            P1_load(nxt)
            P1_l1(nxt)
        if 0 <= s - 2 < M:
            P6(SEQ[s - 2])
            glue_pe(SEQ[s - 2])
        if cur:
            P2_chunk(cur, range(0, 4))
        if nxt:
            P1_l2(nxt)
        if cur:
            P2_chunk(cur, range(4, 7))
        if nxt:
            P1_l3V(nxt)
        if 0 <= s - 1 < M:
            P4(SEQ[s - 1])
            P5(SEQ[s - 1])
        if cur:
            P3(cur)
        if 0 <= s - 2 < M:
            glue_rest(SEQ[s - 2])


def build_nc():
    nc = bacc.Bacc()
    P = {}

    def dp(name, shape, dtype=F32, out=False):
        P[name] = nc.declare_dram_parameter(name, list(shape), dtype, isOutput=out)

    dp("xsn", (BL, 128, NT * XW), BF16)
    dp("csT", (BL, C, N), BF16)
    dp("stateT", (BL, DOUT, N), BF16)
    dp("xT", (BL, DIN, N), BF16)
    dp("tdso4", (BL, E, 4 * NP), BF16)
    dp("embT", (E, N), BF16)
    dp("embSS", (E * 128, N), BF16)
    dp("embX40", (40, N), BF16)
    dp("sel4", (4, 40), BF16)
    dp("wzs_g", (E * 128, OG), BF16)
    dp("wzs_u", (E * 128, OU), BF16)
    dp("wx_g", (40, OG), BF16)
    dp("wx_u", (40, OU), BF16)
    dp("bpool_g", (E, OG), BF16)
    dp("bpool_u", (E, OU), BF16)
    for br in ("g", "u"):
        dp(f"fc1w_{br}", (C, 16), BF16)
        dp(f"fc2w_{br}", (16, 2), BF16)
        dp(f"fc3w_{br}", (2, E), BF16)
        dp(f"fcb1_{br}", (16, 1))
        dp(f"fcb2_{br}", (2, 1))
        dp(f"fcb3_{br}", (E, 1))
    dp("ident", (128, 128))
    dp("identB64", (128, 64), BF16)
    dp("out", (BL, OU, N), BF16, out=True)
    with tile.TileContext(nc) as tc:
        with ExitStack() as ctx:
            _build_body(tc, ctx, nc, P)
    nc.finalize()
    return nc


_NC_CACHE = {}


def _get_nc():
    if "nc" not in _NC_CACHE:
        _NC_CACHE["nc"] = build_nc()
    return _NC_CACHE["nc"]


def _make_in_maps(inputs):
    f32 = lambda a: np.ascontiguousarray(a, dtype=np.float32)
    bf = lambda a: np.ascontiguousarray(np.asarray(a, dtype=np.float32).astype(BF16_NP))
    x = f32(inputs["x"])
    state = f32(inputs["state"])
    emb = f32(inputs["node_embeddings"])
    time, day = f32(inputs["time"]), f32(inputs["day"])
    speed, occupy = f32(inputs["speed"]), f32(inputs["occupy"])

    embT = emb.T                                      # (E, N)
    embSS = np.repeat(embT[:, None, :], 128, axis=1).reshape(E * 128, N)
    embX40 = np.repeat(embT[:, None, :], 4, axis=1).reshape(E * 4, N)
    sel4 = np.tile(np.eye(4, dtype=np.float32), (1, E))  # (4, 40)
    perm_feat = list(range(DIN, C)) + [0, 1]          # state-first

    def pack_w(wpool, operm):
        wp = wpool[..., operm]
        wzs = np.concatenate([wp[:, 1, DIN:, :], wp[:, 0, DIN:, :]], axis=1)
        wzs = wzs.reshape(E * 128, -1)
        wxp = np.stack([wp[:, 1, 0, :], wp[:, 1, 1, :],
                        wp[:, 0, 0, :], wp[:, 0, 1, :]], axis=1)
        wxp = wxp.reshape(E * 4, -1)
        return wzs, wxp

    operm_g = list(range(DOUT, OG)) + list(range(DOUT))  # [r; z]
    wzs_g, wx_g = pack_w(inputs["gate_wpool"], operm_g)
    wzs_u, wx_u = pack_w(inputs["update_wpool"], list(range(OU)))

    identB64 = np.zeros((128, 64), np.float32)
    identB64[64:128, :] = np.eye(64, dtype=np.float32)

    shared = {
        "embT": bf(embT),
        "embSS": bf(embSS),
        "embX40": bf(embX40),
        "sel4": bf(sel4),
        "wzs_g": bf(wzs_g),
        "wzs_u": bf(wzs_u),
        "wx_g": bf(wx_g),
        "wx_u": bf(wx_u),
        "bpool_g": bf(inputs["gate_bpool"][:, operm_g]),
        "bpool_u": bf(inputs["update_bpool"]),
        "ident": np.eye(128, dtype=np.float32),
        "identB64": bf(identB64),
    }
    for br, pre in (("g", "gate"), ("u", "update")):
        shared[f"fc1w_{br}"] = bf(inputs[f"{pre}_fc1_w"][perm_feat, :])
        shared[f"fc2w_{br}"] = bf(inputs[f"{pre}_fc2_w"])
        shared[f"fc3w_{br}"] = bf(inputs[f"{pre}_fc3_w"])
        shared[f"fcb1_{br}"] = f32(inputs[f"{pre}_fc1_b"].reshape(16, 1))
        shared[f"fcb2_{br}"] = f32(inputs[f"{pre}_fc2_b"].reshape(2, 1))
        shared[f"fcb3_{br}"] = f32(inputs[f"{pre}_fc3_b"].reshape(E, 1))

    in_maps = []
    for c in range(NCORES):
        sl = slice(c * BL, (c + 1) * BL)
        m = dict(shared)
        xs, ss = x[sl], state[sl]
        # packed natural xs: (BL, 128, NT*XW); tile nt at cols nt*XW..
        xsn = np.zeros((BL, 128, NT * XW), np.float32)
        for nt in range(NT):
            p = _pt(nt)
            xsn[:, :p, nt * XW:nt * XW + DOUT] = ss[:, nt * 128:nt * 128 + p, :]
            xsn[:, :p, nt * XW + DOUT:nt * XW + C] = xs[:, nt * 128:nt * 128 + p, :]
        m["xsn"] = bf(xsn)
        m["csT"] = bf(np.concatenate([ss.transpose(0, 2, 1),
                                      xs.transpose(0, 2, 1)], axis=1))
        m["stateT"] = bf(ss.transpose(0, 2, 1))
        m["xT"] = bf(xs.transpose(0, 2, 1))
        # packed time/day/speed/occupy: (BL, E, 4*NP)
        td4 = np.zeros((BL, E, 4 * NP), np.float32)
        for j, a in enumerate((time, day, speed, occupy)):
            td4[:, :, j * NP:j * NP + N] = a[sl].transpose(0, 2, 1)
        m["tdso4"] = bf(td4)
        in_maps.append(m)
    return in_maps


def _run(inputs, trace=False):
    nc = _get_nc()
    in_maps = _make_in_maps(inputs)
    res = run_bass_kernel_spmd(nc, in_maps, core_ids=list(range(NCORES)), trace=trace)
    out = np.concatenate(
        [np.asarray(res.results[i]["out"]).astype(np.float32).transpose(0, 2, 1)
         for i in range(NCORES)],
        axis=0,
    )
    return np.ascontiguousarray(out), res


def kernel(**inputs):
    out, _ = _run(inputs, trace=False)
    return out
